# revision 1
# baseline (speedup 1.0000x reference)
"""Grouped multi-query attention (group axis summed) on 8 trn2 NeuronCores.

Math: reference sums the g axis of the grouped Q heads inside the score
einsum, so the whole module collapses to standard 8-head attention with
W_Qeff[n] = sum_g W_Q[4n+g] (and the 1/sqrt(64) score scale folded in).

Sharding: core c -> (batch b = c//2, kv-head half = c%2). Each core runs
4 heads of one batch and produces a full [2048, 2048] partial of the
output projection; the host sums the two halves per batch.

Per-core layout (all matmuls are out = lhsT.T @ rhs, bf16 in / f32 acc):
  xT [d, t] host-pretransposed; Q_T/K_T [2*64 head-pair rows, t] built by
  projection matmuls; V [t, 4*64] built directly; scores computed
  transposed S_T[k, q] = K @ Q^T so softmax/PV need no transposes at all;
  per-head Z lands in psum partitions 64*rj..64*rj+63 via tile_position
  col offsets; softmax denominators come from a packed M=1 ones-matmul,
  extracted across partitions with a tiny DMA, inverted, and broadcast
  back over 64 partitions with a K=1 ones-matmul. The two heads of a pair
  run concurrently in disjoint PE row/col groups (tile_position packing);
  causal masking is a multiplicative bf16 {0,1} DVE mult on the exp'd
  tile (4x mode); out-projection is interleaved per q-block round.
"""

import numpy as np

S = 2048
D = 2048
HD = 64
NKV = 8
GQ = 4  # grouped q heads per kv head (summed)
HPC = 4  # heads per core
TB = 512
QB = 512
NTB = S // TB
NDC = D // 128
NQB = S // QB
NKT = S // 128
IGNORE = -100000.0

_CACHE = {}


def _build_program():
    import concourse.bass as bass
    import concourse.tile as tile
    from concourse import bacc, mybir

    F32 = mybir.dt.float32
    BF16 = mybir.dt.bfloat16
    AF = mybir.ActivationFunctionType

    nc = bacc.Bacc("TRN2", target_bir_lowering=False, debug=False,
                   enable_asserts=False, num_devices=8)

    xT_d = nc.dram_tensor("xT", [D, S], BF16, kind="ExternalInput").ap()
    wq_d = nc.dram_tensor("wq", [D, HPC * HD], BF16, kind="ExternalInput").ap()
    wk_d = nc.dram_tensor("wk", [D, HPC * HD], BF16, kind="ExternalInput").ap()
    wv_d = nc.dram_tensor("wv", [D, HPC * HD], BF16, kind="ExternalInput").ap()
    wo_d = nc.dram_tensor("wo", [HPC * HD, D], BF16, kind="ExternalInput").ap()
    mask_d = nc.dram_tensor("mask", [128, 4, QB], BF16, kind="ExternalInput").ap()
    o_d = nc.dram_tensor("o", [S, D], F32, kind="ExternalOutput").ap()

    def r(ap):
        return ap

    import contextlib
    with tile.TileContext(nc) as tc, \
            nc.allow_low_precision(reason="bf16 matmul operands by design"):
        with (
            tc.tile_pool(name="singles", bufs=1) as singles,
            tc.tile_pool(name="persist", bufs=1) as persist,
            tc.tile_pool(name="work", bufs=4) as work,
            tc.tile_pool(name="outsb", bufs=3) as outsb,
            tc.tile_pool(name="tiny", bufs=4) as tiny,
            tc.tile_pool(name="bcsb", bufs=2) as bcsb,
        ):
            # constants + weights
            wq_sb = singles.tile([128, NDC, HPC * HD], BF16, tag="wq")
            wk_sb = singles.tile([128, NDC, HPC * HD], BF16, tag="wk")
            wv_sb = singles.tile([128, NDC, HPC * HD], BF16, tag="wv")
            wo_sb = singles.tile([128, 2, D], BF16, tag="wo")
            mask_sb = singles.tile([128, 4, QB], BF16, tag="mask")
            ones_col = singles.tile([128, 1], BF16, tag="onec")
            ones_row = singles.tile([1, HD], BF16, tag="oner")

            nc.sync.dma_start(out=wq_sb, in_=wq_d.rearrange("(c p) n -> p c n", p=128))
            nc.sync.dma_start(out=wk_sb, in_=wk_d.rearrange("(c p) n -> p c n", p=128))
            nc.sync.dma_start(out=wv_sb, in_=wv_d.rearrange("(c p) n -> p c n", p=128))
            nc.sync.dma_start(out=wo_sb, in_=wo_d.rearrange("(g p) d -> p g d", p=128))
            nc.sync.dma_start(out=mask_sb, in_=mask_d)
            nc.vector.memset(ones_col, 1.0)
            nc.vector.memset(ones_row, 1.0)

            qT_sb = persist.tile([128, 2, S], BF16, tag="qT")
            kT_sb = persist.tile([128, 2, S], BF16, tag="kT")
            v_sb = persist.tile([128, NKT, HPC * HD], BF16, tag="v")
            z_sb = persist.tile([128, 2, S], BF16, tag="z")

            # ---- phase 1: projections ----
            with tc.tile_pool(name="ph1ps", bufs=4,
                              space=bass.MemorySpace.PSUM) as ph1ps:
                for tb in range(NTB):
                    ps_q = [ph1ps.tile([128, TB], F32, tag="qk", name=f"psq{tb}_{i}") for i in range(2)]
                    ps_k = [ph1ps.tile([128, TB], F32, tag="qk", name=f"psk{tb}_{i}") for i in range(2)]
                    ps_v = [ph1ps.tile([128, HPC * HD], F32, tag="v", name=f"psv{tb}_{i}") for i in range(4)]
                    for dc in range(NDC):
                        xt = work.tile([128, TB], BF16, tag="xt")
                        nc.sync.dma_start(
                            out=xt,
                            in_=xT_d[dc * 128:(dc + 1) * 128, tb * TB:(tb + 1) * TB])
                        st = dict(start=(dc == 0), stop=(dc == NDC - 1))
                        for g in range(2):
                            nc.tensor.matmul(
                                ps_q[g], r(wq_sb[:, dc, 128 * g:128 * (g + 1)]),
                                r(xt), **st)
                            nc.tensor.matmul(
                                ps_k[g], r(wk_sb[:, dc, 128 * g:128 * (g + 1)]),
                                r(xt), **st)
                        for tt in range(4):
                            nc.tensor.matmul(
                                ps_v[tt], r(xt[:, tt * 128:(tt + 1) * 128]),
                                r(wv_sb[:, dc, :]), **st)
                    for g in range(2):
                        nc.scalar.copy(out=qT_sb[:, g, tb * TB:(tb + 1) * TB], in_=ps_q[g])
                        nc.scalar.copy(out=kT_sb[:, g, tb * TB:(tb + 1) * TB], in_=ps_k[g])
                    for tt in range(4):
                        nc.vector.tensor_copy(
                            out=v_sb[:, 4 * tb + tt, :], in_=ps_v[tt])

            # ---- phase 2: attention, phase 3: output projection ----
            with (
                tc.tile_pool(name="sps", bufs=2, space=bass.MemorySpace.PSUM) as sps,
                tc.tile_pool(name="zps", bufs=2, space=bass.MemorySpace.PSUM) as zps,
                tc.tile_pool(name="smps", bufs=1, space=bass.MemorySpace.PSUM) as smps,
                tc.tile_pool(name="bcps", bufs=1, space=bass.MemorySpace.PSUM) as bcps,
                tc.tile_pool(name="ops", bufs=2, space=bass.MemorySpace.PSUM) as ops,
            ):
                for jq in range(NQB):
                    nkt = 4 * (jq + 1)
                    for g in range(2):
                        # both heads of pair g packed into disjoint array
                        # regions: jj=0 -> z rows 64:128, sums row 32;
                        # jj=1 -> z rows 0:64, sums row 64
                        ps_z = zps.tile([128, QB], F32, tag="z",
                                        name=f"z{jq}_{g}")
                        ps_sm = smps.tile([128, QB], F32, tag="sm",
                                          name=f"sm{jq}_{g}")
                        for ik in range(nkt):
                            p2 = []
                            for jj in range(2):
                                ps_s = sps.tile([128, QB], F32, tag="s",
                                                name=f"s{jq}_{g}_{ik}_{jj}")
                                nc.tensor.matmul(
                                    ps_s,
                                    kT_sb[64 * jj:64 * (jj + 1), g,
                                          ik * 128:(ik + 1) * 128],
                                    qT_sb[64 * jj:64 * (jj + 1), g,
                                          jq * QB:(jq + 1) * QB],
                                    start=True, stop=True)
                                p_sb = work.tile([128, QB], BF16, tag="p",
                                                 name=f"p{jq}_{g}_{ik}_{jj}")
                                nc.scalar.activation(out=p_sb, in_=ps_s,
                                                     func=AF.Exp)
                                if ik >= 4 * jq:
                                    nc.vector.tensor_mul(
                                        p_sb, p_sb, mask_sb[:, ik - 4 * jq, :])
                                p2.append(p_sb)
                            st = dict(start=(ik == 0), stop=(ik == nkt - 1),
                                      skip_group_check=True)
                            for jj, p_sb in enumerate(p2):
                                rj = 1 - jj
                                nc.tensor.matmul(
                                    ps_z[64 * rj:64 * (rj + 1), :],
                                    v_sb[:, ik, HD * (2 * g + jj):
                                         HD * (2 * g + jj + 1)],
                                    p_sb, tile_position=(0, 64 * rj), **st)
                            for jj, p_sb in enumerate(p2):
                                sc = 32 if jj == 0 else 64
                                nc.tensor.matmul(
                                    ps_sm[sc:sc + 1, :], ones_col, p_sb,
                                    tile_position=(0, sc), **st)
                        for jj in range(2):
                            rj = 1 - jj
                            sc = 32 if jj == 0 else 64
                            sumhi = tiny.tile([65, QB], F32, tag="sumhi",
                                              name=f"sh{jq}_{g}_{jj}")
                            nc.scalar.copy(out=sumhi[sc:sc + 1, :],
                                           in_=ps_sm[sc:sc + 1, :])
                            sums_sb = tiny.tile([1, QB], F32, tag="sums",
                                                name=f"su{jq}_{g}_{jj}")
                            nc.gpsimd.dma_start(out=sums_sb,
                                                in_=sumhi[sc:sc + 1, :])
                            recip_sb = tiny.tile([1, QB], BF16, tag="recip",
                                                 name=f"re{jq}_{g}_{jj}")
                            nc.vector.reciprocal(out=recip_sb, in_=sums_sb)
                            ps_bc = bcps.tile([128, QB], F32, tag="bc",
                                              name=f"bc{jq}_{g}_{jj}")
                            nc.tensor.matmul(
                                ps_bc[64 * rj:64 * (rj + 1), :], ones_row,
                                recip_sb, tile_position=(0, 64 * rj),
                                start=True, stop=True)
                            bc_sb = bcsb.tile([128, QB], F32, tag="bc",
                                              name=f"bs{jq}_{g}_{jj}")
                            nc.scalar.copy(
                                out=bc_sb[64 * rj:64 * (rj + 1), :],
                                in_=ps_bc[64 * rj:64 * (rj + 1), :])
                            nc.vector.tensor_mul(
                                z_sb[64 * rj:64 * (rj + 1), g,
                                     jq * QB:(jq + 1) * QB],
                                ps_z[64 * rj:64 * (rj + 1), :],
                                bc_sb[64 * rj:64 * (rj + 1), :])

                    for it in range(4 * jq, 4 * jq + 4):
                        for db in range(4):
                            ps_o = ops.tile([128, 512], F32, tag="o",
                                            name=f"o{it}_{db}")
                            for g in range(2):
                                nc.tensor.matmul(
                                    ps_o, z_sb[:, g, it * 128:(it + 1) * 128],
                                    wo_sb[:, g, db * 512:(db + 1) * 512],
                                    start=(g == 0), stop=(g == 1))
                            o_sb = outsb.tile([128, 512], F32, tag="o",
                                              name=f"os{it}_{db}")
                            nc.vector.tensor_copy(out=o_sb, in_=ps_o)
                            nc.sync.dma_start(
                                out=o_d[it * 128:(it + 1) * 128,
                                        db * 512:(db + 1) * 512],
                                in_=o_sb)

    nc.compile()
    return nc


def get_program():
    if "nc" not in _CACHE:
        _CACHE["nc"] = _build_program()
    return _CACHE["nc"]


def make_in_maps(normalized_resid_pre, W_Q, W_K, W_V, W_O):
    x = normalized_resid_pre
    x = np.ascontiguousarray(np.asarray(x, np.float32))
    W_Q = np.asarray(W_Q, np.float32)
    W_K = np.asarray(W_K, np.float32)
    W_V = np.asarray(W_V, np.float32)
    W_O = np.asarray(W_O, np.float32)
    wqe = W_Q.reshape(NKV, GQ, D, HD).sum(1) * (1.0 / np.sqrt(HD))

    kk = np.arange(128)[:, None, None]
    mm = np.arange(4)[None, :, None]
    qq = np.arange(QB)[None, None, :]
    import ml_dtypes
    mask = np.where(mm * 128 + kk <= qq, 1.0, 0.0).astype(ml_dtypes.bfloat16)
    mask = np.ascontiguousarray(mask)

    in_maps = []
    for c in range(8):
        b, half = divmod(c, 2)
        heads = [4 * half + m for m in range(HPC)]
        xT = np.ascontiguousarray(x[b].T)
        wq = np.ascontiguousarray(np.concatenate([wqe[n] for n in heads], 1))
        wk = np.ascontiguousarray(np.concatenate([W_K[n] for n in heads], 1))
        wv = np.ascontiguousarray(np.concatenate([W_V[n] for n in heads], 1))
        # z rows within pair g: [0:64] = head 2g+1, [64:128] = head 2g
        wo = np.ascontiguousarray(np.concatenate(
            [W_O[heads[1]], W_O[heads[0]], W_O[heads[3]], W_O[heads[2]]], 0))
        import ml_dtypes
        bf = ml_dtypes.bfloat16
        in_maps.append({"xT": xT.astype(bf), "wq": wq.astype(bf),
                        "wk": wk.astype(bf), "wv": wv.astype(bf),
                        "wo": wo.astype(bf), "mask": mask})
    return in_maps


def run(in_maps, **kw):
    from concourse.bass_utils import run_bass_kernel_spmd
    return run_bass_kernel_spmd(get_program(), in_maps,
                                core_ids=list(range(8)), **kw)


def kernel(normalized_resid_pre, W_Q, W_K, W_V, W_O):
    in_maps = make_in_maps(normalized_resid_pre, W_Q, W_K, W_V, W_O)
    res = run(in_maps)
    out = np.empty((4, S, D), np.float32)
    for b in range(4):
        out[b] = res.results[2 * b]["o"] + res.results[2 * b + 1]["o"]
    return out



# revision 5
# speedup vs baseline: 1.3313x; 1.3313x over previous
"""Grouped multi-query attention (group axis summed) on 8 trn2 NeuronCores.

Math: reference sums the g axis of the grouped Q heads inside the score
einsum, so the whole module collapses to standard 8-head attention with
W_Qeff[n] = sum_g W_Q[4n+g] (and the 1/sqrt(64) score scale folded in).

Sharding: core c -> (batch b = c//2, kv-head half = c%2). Each core runs
4 heads of one batch and produces a full [2048, 2048] partial of the
output projection; the host sums the two halves per batch.

Per-core pipeline (all matmuls are out = lhsT.T @ rhs, bf16 in / f32 acc;
cost model charges ~N_rhs_cols cycles per matmul, Ldweights free):
  scores S_T[k,q] = K @ Q^T per head (contract hd=64), causally trimmed;
  exp on Act; diagonal 128x128 subtiles masked multiplicatively on DVE;
  PV computed TRANSPOSED: z_T[q,hd] = P^T V with the p subtile as lhsT
  (rhs = 64-col V slice, 8x cheaper than streaming p), softmax
  denominators as 1-col P^T @ ones matmuls (~free) accumulated in a
  shared psum bank; normalization is a per-partition tensor_scalar_mul
  (fast direction); z_T -> z via identity-matmul transposes; output
  projection contracts head pairs (K=128). Projections, attention and
  the (deferrable) out-projection are software-pipelined so PE never
  starves while Act runs the big exp tail.
"""

import numpy as np

S = 2048
D = 2048
HD = 64
NKV = 8
GQ = 4    # grouped q heads per kv head (summed)
HPC = 4   # heads per core
QB = 512  # q block
NDC = D // 128
NQB = S // QB
NKT = S // 128
LAG = 2   # PV lags scores by this many k-tiles

_CACHE = {}


def _build_program():
    import concourse.bass as bass
    import concourse.tile as tile
    from concourse import bacc, mybir

    F32 = mybir.dt.float32
    BF16 = mybir.dt.bfloat16
    AF = mybir.ActivationFunctionType

    nc = bacc.Bacc("TRN2", target_bir_lowering=False, debug=False,
                   enable_asserts=False, num_devices=8)

    xT_d = nc.dram_tensor("xT", [D, S], BF16, kind="ExternalInput").ap()
    wq_d = nc.dram_tensor("wq", [D, HPC * HD], BF16, kind="ExternalInput").ap()
    wk_d = nc.dram_tensor("wk", [D, HPC * HD], BF16, kind="ExternalInput").ap()
    wv_d = nc.dram_tensor("wv", [D, HPC * HD], BF16, kind="ExternalInput").ap()
    wo_d = nc.dram_tensor("wo", [HPC * HD, D], BF16, kind="ExternalInput").ap()
    mask_d = nc.dram_tensor("mask", [128, 128], BF16, kind="ExternalInput").ap()
    id_d = nc.dram_tensor("iden", [128, 128], BF16, kind="ExternalInput").ap()
    o_d = nc.dram_tensor("o", [S, D], F32, kind="ExternalOutput").ap()

    with tile.TileContext(nc) as tc, \
            nc.allow_low_precision(reason="bf16 matmul operands by design"):
        with (
            tc.tile_pool(name="singles", bufs=1) as singles,
            tc.tile_pool(name="persist", bufs=1) as persist,
            tc.tile_pool(name="xtp", bufs=32) as xtp,
            tc.tile_pool(name="pp", bufs=10) as pp,
            tc.tile_pool(name="ztp", bufs=2) as ztp,
            tc.tile_pool(name="outsb", bufs=3) as outsb,
            tc.tile_pool(name="tiny", bufs=2) as tiny,
            tc.tile_pool(name="gemm", bufs=3,
                         space=bass.MemorySpace.PSUM) as gemm,
            tc.tile_pool(name="sps", bufs=2,
                         space=bass.MemorySpace.PSUM) as sps,
            tc.tile_pool(name="zps", bufs=2,
                         space=bass.MemorySpace.PSUM) as zps,
            tc.tile_pool(name="dentp", bufs=1,
                         space=bass.MemorySpace.PSUM) as dentp,
        ):
            # ---- constants + weights ----
            wq_sb = singles.tile([128, NDC, HPC * HD], BF16, tag="wq")
            wk_sb = singles.tile([128, NDC, HPC * HD], BF16, tag="wk")
            wv_sb = singles.tile([128, NDC, HPC * HD], BF16, tag="wv")
            wo_sb = singles.tile([128, 2, D], BF16, tag="wo")
            mask_sb = singles.tile([128, 128], BF16, tag="mask")
            id_sb = singles.tile([128, 128], BF16, tag="iden")
            ones_col = singles.tile([128, 1], BF16, tag="onec")

            nc.sync.dma_start(out=wq_sb, in_=wq_d.rearrange("(c p) n -> p c n", p=128))
            nc.sync.dma_start(out=wk_sb, in_=wk_d.rearrange("(c p) n -> p c n", p=128))
            nc.sync.dma_start(out=wv_sb, in_=wv_d.rearrange("(c p) n -> p c n", p=128))
            nc.sync.dma_start(out=wo_sb, in_=wo_d.rearrange("(g p) d -> p g d", p=128))
            nc.sync.dma_start(out=mask_sb, in_=mask_d)
            nc.sync.dma_start(out=id_sb, in_=id_d)
            nc.vector.memset(ones_col, 1.0)

            qT_sb = persist.tile([128, 2, S], BF16, tag="qT")
            kT_sb = persist.tile([128, 2, S], BF16, tag="kT")
            v_sb = persist.tile([128, NKT, HPC * HD], BF16, tag="v")
            z_sb = persist.tile([128, 2, S], BF16, tag="z")

            xt_tiles = {}

            def load_xt(tb):
                for dc in range(NDC):
                    t = xtp.tile([128, QB], BF16, tag="xt",
                                 name=f"xt{tb}_{dc}")
                    nc.sync.dma_start(
                        out=t,
                        in_=xT_d[dc * 128:(dc + 1) * 128,
                                 tb * QB:(tb + 1) * QB])
                    xt_tiles[(tb, dc)] = t

            # ---- projection groups (each = 1 psum bank accumulated over dc)
            def proj_group_qk(tb, g, w_sb, dst_sb):
                ps = gemm.tile([128, QB], F32, tag="gemm",
                               name=f"pqk{tb}_{g}_{id(w_sb)}")
                for dc in range(NDC):
                    nc.tensor.matmul(
                        ps, w_sb[:, dc, 128 * g:128 * (g + 1)],
                        xt_tiles[(tb, dc)],
                        start=(dc == 0), stop=(dc == NDC - 1))
                nc.scalar.copy(out=dst_sb[:, g, tb * QB:(tb + 1) * QB],
                               in_=ps)

            def proj_group_v(tb, half):
                ps = gemm.tile([128, QB], F32, tag="gemm",
                               name=f"pv{tb}_{half}")
                for dc in range(NDC):
                    for ti in range(2):
                        tt = 2 * half + ti
                        nc.tensor.matmul(
                            ps[:, 256 * ti:256 * (ti + 1)],
                            xt_tiles[(tb, dc)][:, 128 * tt:128 * (tt + 1)],
                            wv_sb[:, dc, :],
                            start=(dc == 0 and ti == 0),
                            stop=(dc == NDC - 1),
                            skip_group_check=True)
                nc.vector.tensor_copy(
                    out=v_sb[:, 4 * tb + 2 * half:4 * tb + 2 * half + 2, :],
                    in_=ps)

            def proj_steps(tb):
                """Generator: 6 psum-bank groups for token block tb."""
                proj_group_qk(tb, 0, wq_sb, qT_sb)
                yield
                proj_group_qk(tb, 1, wq_sb, qT_sb)
                yield
                proj_group_qk(tb, 0, wk_sb, kT_sb)
                yield
                proj_group_qk(tb, 1, wk_sb, kT_sb)
                yield
                proj_group_v(tb, 0)
                yield
                proj_group_v(tb, 1)
                yield

            # ---- attention ----
            def emit_pv(jq, ik, plist, ps_z, dent):
                for (h, qc0, p_sb) in plist:
                    g, jj = divmod(h, 2)
                    for qs in range(qc0 // 128, 4):
                        # start=True only on the FIRST matmul into each psum
                        # bank: it marks the whole bank pending-zero, so each
                        # other subregion's first touch zero-fills correctly.
                        # A start on a later group would wipe live neighbors.
                        last = (ik == 4 * jq + qs)
                        nc.tensor.matmul(
                            ps_z[g][:, jj * 256 + qs * 64:
                                    jj * 256 + (qs + 1) * 64],
                            p_sb[:, qs * 128:(qs + 1) * 128],
                            v_sb[:, ik, h * 64:(h + 1) * 64],
                            start=(ik == 0 and jj == 0 and qs == 0),
                            stop=last,
                            skip_group_check=True)
                        nc.tensor.matmul(
                            dent[:, 4 * h + qs:4 * h + qs + 1],
                            p_sb[:, qs * 128:(qs + 1) * 128],
                            ones_col,
                            start=(ik == 0 and h == 0 and qs == 0),
                            stop=last,
                            skip_group_check=True)

            def attn_steps(jq):
                nkt = 4 * (jq + 1)
                ps_z = [zps.tile([128, QB], F32, tag="z", name=f"z{jq}_{g}")
                        for g in range(2)]
                dent = dentp.tile([128, QB], F32, tag="dent",
                                  name=f"dent{jq}")
                pend = []
                for ik in range(nkt):
                    plist = []
                    for h in range(HPC):
                        g, jj = divmod(h, 2)
                        qc0 = max(0, ik - 4 * jq) * 128
                        ps_s = sps.tile([128, QB], F32, tag="s",
                                        name=f"s{jq}_{ik}_{h}")
                        nc.tensor.matmul(
                            ps_s[:, qc0:],
                            kT_sb[64 * jj:64 * (jj + 1), g,
                                  ik * 128:(ik + 1) * 128],
                            qT_sb[64 * jj:64 * (jj + 1), g,
                                  jq * QB + qc0:(jq + 1) * QB],
                            start=True, stop=True)
                        p_sb = pp.tile([128, QB], BF16, tag="p",
                                       name=f"p{jq}_{ik}_{h}")
                        nc.scalar.activation(out=p_sb[:, qc0:],
                                             in_=ps_s[:, qc0:], func=AF.Exp)
                        if ik >= 4 * jq:
                            nc.vector.tensor_mul(
                                p_sb[:, qc0:qc0 + 128],
                                p_sb[:, qc0:qc0 + 128], mask_sb)
                        plist.append((h, qc0, p_sb))
                    pend.append((ik, plist))
                    yield
                    if len(pend) > LAG:
                        ik0, pl0 = pend.pop(0)
                        emit_pv(jq, ik0, pl0, ps_z, dent)
                        yield
                while pend:
                    ik0, pl0 = pend.pop(0)
                    emit_pv(jq, ik0, pl0, ps_z, dent)
                    yield
                # finalize: recip, normalize, transpose into z_sb
                recip = tiny.tile([128, 16], F32, tag="recip",
                                  name=f"re{jq}")
                nc.vector.reciprocal(out=recip, in_=dent[:, 0:16])
                zt = ztp.tile([128, 4, 256], BF16, tag="zt",
                              name=f"zt{jq}")
                for h in range(HPC):
                    g, jj = divmod(h, 2)
                    for qs in range(4):
                        nc.vector.tensor_scalar_mul(
                            zt[:, qs, 64 * h:64 * (h + 1)],
                            ps_z[g][:, jj * 256 + qs * 64:
                                    jj * 256 + (qs + 1) * 64],
                            recip[:, 4 * h + qs:4 * h + qs + 1])
                yield
                for qs in range(4):
                    for g in range(2):
                        slot = (qs * 2 + g) % 3
                        tp = dent[:, 128 * (slot + 1):128 * (slot + 2)]
                        nc.tensor.matmul(
                            tp, zt[:, qs, 128 * g:128 * (g + 1)], id_sb,
                            start=True, stop=True, skip_group_check=True)
                        nc.vector.tensor_copy(
                            out=z_sb[:, g,
                                     jq * QB + qs * 128:jq * QB
                                     + (qs + 1) * 128],
                            in_=tp)
                    yield

            # ---- output projection ----
            def op_tile(it, db):
                ps = gemm.tile([128, QB], F32, tag="gemm",
                               name=f"op{it}_{db}")
                for g in range(2):
                    nc.tensor.matmul(
                        ps, z_sb[:, g, it * 128:(it + 1) * 128],
                        wo_sb[:, g, db * QB:(db + 1) * QB],
                        start=(g == 0), stop=(g == 1))
                o_sb = outsb.tile([128, QB], F32, tag="o",
                                  name=f"os{it}_{db}")
                nc.vector.tensor_copy(out=o_sb, in_=ps)
                nc.sync.dma_start(
                    out=o_d[it * 128:(it + 1) * 128,
                            db * QB:(db + 1) * QB],
                    in_=o_sb)

            def op_steps(jq):
                for it in range(4 * jq, 4 * jq + 4):
                    for db in range(4):
                        op_tile(it, db)
                        yield

            def interleave(*gens):
                """Round-robin, weighted so all generators finish together."""
                gens = [g for g in gens if g is not None]
                active = [iter(g) for g in gens]
                while active:
                    nxt = []
                    for g in active:
                        try:
                            next(g)
                            nxt.append(g)
                        except StopIteration:
                            pass
                    active = nxt

            # ---- schedule ----
            load_xt(0)
            for _ in proj_steps(0):
                pass
            load_xt(1)
            interleave(attn_steps(0), proj_steps(1))
            load_xt(2)
            interleave(attn_steps(1), proj_steps(2), op_steps(0))
            load_xt(3)
            interleave(attn_steps(2), proj_steps(3), op_steps(1))
            interleave(attn_steps(3), op_steps(2))
            for _ in op_steps(3):
                pass

    nc.compile()
    return nc


def get_program():
    if "nc" not in _CACHE:
        _CACHE["nc"] = _build_program()
    return _CACHE["nc"]


def make_in_maps(normalized_resid_pre, W_Q, W_K, W_V, W_O):
    import ml_dtypes
    bf = ml_dtypes.bfloat16
    x = np.ascontiguousarray(np.asarray(normalized_resid_pre, np.float32))
    W_Q = np.asarray(W_Q, np.float32)
    W_K = np.asarray(W_K, np.float32)
    W_V = np.asarray(W_V, np.float32)
    W_O = np.asarray(W_O, np.float32)
    wqe = W_Q.reshape(NKV, GQ, D, HD).sum(1) * (1.0 / np.sqrt(HD))

    kk = np.arange(128)[:, None]
    qq = np.arange(128)[None, :]
    mask = np.ascontiguousarray((kk <= qq).astype(bf))
    iden = np.ascontiguousarray(np.eye(128).astype(bf))

    in_maps = []
    for c in range(8):
        b, half = divmod(c, 2)
        heads = [4 * half + m for m in range(HPC)]
        xT = np.ascontiguousarray(x[b].T)
        wq = np.ascontiguousarray(np.concatenate([wqe[n] for n in heads], 1))
        wk = np.ascontiguousarray(np.concatenate([W_K[n] for n in heads], 1))
        wv = np.ascontiguousarray(np.concatenate([W_V[n] for n in heads], 1))
        wo = np.ascontiguousarray(np.concatenate([W_O[n] for n in heads], 0))
        in_maps.append({"xT": xT.astype(bf), "wq": wq.astype(bf),
                        "wk": wk.astype(bf), "wv": wv.astype(bf),
                        "wo": wo.astype(bf), "mask": mask, "iden": iden})
    return in_maps


def run(in_maps, **kw):
    from concourse.bass_utils import run_bass_kernel_spmd
    return run_bass_kernel_spmd(get_program(), in_maps,
                                core_ids=list(range(8)), **kw)


def kernel(normalized_resid_pre, W_Q, W_K, W_V, W_O):
    in_maps = make_in_maps(normalized_resid_pre, W_Q, W_K, W_V, W_O)
    res = run(in_maps)
    out = np.empty((4, S, D), np.float32)
    for b in range(4):
        out[b] = res.results[2 * b]["o"] + res.results[2 * b + 1]["o"]
    return out
